# revision 5
# baseline (speedup 1.0000x reference)
"""Grouped MLP (MoE expert MLP, ragged token groups) on 8 TRN2 NeuronCores.

Strategy: token-parallel across 4 core-pairs; tensor-parallel (split of
the intermediate dim F) within each pair. Tokens are grouped contiguously
by expert; each expert's group is padded to a multiple of 256 tokens
("units"). Units are assigned to the 4 pairs by a greedy scheduler that
produces a single uniform slot pattern (slot = run of units computed with
one expert's weights) so all 8 cores run ONE fully static SPMD program:

  for slot s (static):  DMA this slot's expert weights (expert id is
      runtime data via a dynamic DRAM offset; weight pools are
      double-buffered so slot s+1 prefetches during slot s compute)
    for body b in slot (static, 512- or 256-token bodies):
      DMA xT [128, 8ht, W] -> fc1 (16 f-tiles x 8 h-tiles matmuls, W wide)
      -> Gelu -> fc2 (8 h-tiles x 16 f-tiles) -> yT fp16 partial -> DMA out

Everything is statically unrolled, so Tile overlaps all DMA with compute;
there are no dynamic-loop all-engine barriers in the steady state (the
only runtime loop is an outer `reps` loop used for timing).

Each core of a pair computes fc1/fc2 over its half of F and writes fp16
partial fc2 sums; the host adds the two halves and scatters to [T, H].
"""

import numpy as np
import ml_dtypes

import concourse.bass as bass
import concourse.mybir as mybir
import concourse.tile as tile
from concourse import bacc
from concourse.bass_utils import run_bass_kernel_spmd

# Problem shape (fixed by the task).
T, H, F, E = 16384, 1024, 4096, 8
NCORES = 8
NPAIRS = 4
UNIT = 256            # token unit (min matmul moving width)
HT = H // 128         # 8 h-tiles
FT2 = F // 2 // 128   # 16 f-tiles per core (half of F)
FH = F // 2           # 2048

_BF16 = mybir.dt.bfloat16
_F16 = mybir.dt.float16
_F32 = mybir.dt.float32
_I32 = mybir.dt.int32

GELU_FUNC = mybir.ActivationFunctionType.Gelu

_cache = {}


def _schedule(counts):
    """counts[E] -> (C, pattern, cells, units_of) for 4 uniform pairs.

    pattern: tuple of slot widths (in units), sum == C
    cells[s][g]: expert id for slot s on pair g
    units_of[g]: list of (expert, tok_start, valid) per chunk j (len C);
                 dummies are (e, -1, 0).
    """
    starts = np.concatenate([[0], np.cumsum(counts)])
    queues = {}
    n = 0
    for e in range(E):
        c = int(counts[e])
        q = []
        for off in range(0, c, UNIT):
            q.append((e, int(starts[e]) + off, min(UNIT, c - off)))
        if q:
            queues[e] = q
            n += len(q)
    if n == 0:
        queues[0] = [(0, -1, 0)]
        n = 1
    C = -(-n // NPAIRS)

    # Greedy slot pattern: each slot assigns (possibly repeated) experts
    # to the 4 pairs; width = what all assigned cells can fill.
    r = {e: len(q) for e, q in queues.items()}
    pattern, cells = [], []
    cap = C
    while cap > 0 and sum(r.values()) > 0:
        order = sorted([e for e in r if r[e] > 0], key=lambda e: -r[e])
        cell = [order[g % len(order)] for g in range(NPAIRS)]
        shares = {}
        for e in cell:
            shares[e] = shares.get(e, 0) + 1
        p = min(max(1, r[e] // shares[e]) for e in set(cell))
        p = max(1, min(p, cap))
        for e in cell:
            r[e] = max(0, r[e] - p)
        pattern.append(p)
        cells.append(cell)
        cap -= p
    if sum(r.values()) > 0:
        # Greedy failed (shouldn't for sane inputs): width-1 fallback.
        pattern, cells = [], []
        r = {e: len(q) for e, q in queues.items()}
        cap = C
        while cap > 0 and sum(r.values()) > 0:
            order = sorted([e for e in r if r[e] > 0], key=lambda e: -r[e])
            cell = [order[g % len(order)] for g in range(NPAIRS)]
            for e in cell:
                r[e] = max(0, r[e] - 1)
            pattern.append(1)
            cells.append(cell)
            cap -= 1
        assert sum(r.values()) == 0, "schedule fallback failed"
    if cap > 0:
        pattern.append(cap)
        cells.append(list(cells[-1]) if cells else [0] * NPAIRS)

    # Deal actual units to (pair, slot) cells.
    pos = {e: 0 for e in queues}
    units_of = [[] for _ in range(NPAIRS)]
    for s, p in enumerate(pattern):
        for g in range(NPAIRS):
            e = cells[s][g]
            for _ in range(p):
                q = queues.get(e, [])
                if pos.get(e, 0) < len(q):
                    units_of[g].append(q[pos[e]])
                    pos[e] += 1
                else:
                    units_of[g].append((e, -1, 0))
    for e, q in queues.items():
        assert pos[e] == len(q), f"unplaced units for expert {e}"
    return sum(pattern), tuple(pattern), cells, units_of


def _bodies(pattern):
    """Slot widths -> list of per-slot body widths (tokens)."""
    out = []
    for p in pattern:
        bl = [2 * UNIT] * (p // 2) + [UNIT] * (p % 2)
        out.append(bl)
    return out


def _build(pattern):
    key = pattern
    if key in _cache:
        return _cache[key]
    C = sum(pattern)
    S = len(pattern)
    bodies = _bodies(pattern)

    nc = bacc.Bacc("TRN2", target_bir_lowering=False, debug=False,
                   num_devices=NCORES)
    xt_d = nc.declare_dram_parameter("xt", [H, C * UNIT], _BF16,
                                     isOutput=False)
    w1_d = nc.declare_dram_parameter("w1", [H, E * FH], _BF16,
                                     isOutput=False)
    w2_d = nc.declare_dram_parameter("w2", [FH, E * H], _BF16,
                                     isOutput=False)
    meta_d = nc.declare_dram_parameter("meta", [1, S + 1], _I32,
                                       isOutput=False)
    yt_d = nc.declare_dram_parameter("yt", [H, C * UNIT], _F16,
                                     isOutput=True)

    xt_r = xt_d.rearrange("(ht p) m -> p ht m", p=128)
    w1_r = w1_d.rearrange("(ht p) m -> p ht m", p=128)
    w2_r = w2_d.rearrange("(ft p) m -> p ft m", p=128)
    yt_r = yt_d.rearrange("(ht p) m -> p ht m", p=128)

    with tile.TileContext(nc) as tc:
        with (
            tc.tile_pool(name="meta", bufs=1) as mpool,
            tc.tile_pool(name="w1", bufs=2) as w1pool,
            tc.tile_pool(name="w2", bufs=2) as w2pool,
            tc.tile_pool(name="x", bufs=2) as xpool,
            tc.tile_pool(name="act", bufs=2) as apool,
            tc.tile_pool(name="y", bufs=2) as ypool,
            tc.tile_pool(name="ps1", bufs=4, space="PSUM") as ps1pool,
            tc.tile_pool(name="ps2", bufs=4, space="PSUM") as ps2pool,
        ):
            mt = mpool.tile([1, S + 1], _I32)
            nc.sync.dma_start(mt[:], meta_d[:])
            reps = nc.values_load(mt[:1, S:S + 1], min_val=1, max_val=100000,
                                  skip_runtime_bounds_check=True)
            w1offs, w2offs = [], []
            for s in range(S):
                # skip_runtime_bounds_check: runtime assert traps kill the
                # axon/PJRT execution path.
                e_s = nc.values_load(mt[:1, s:s + 1], min_val=0,
                                     max_val=E - 1,
                                     skip_runtime_bounds_check=True)
                w1offs.append(nc.s_assert_within(
                    e_s * FH, min_val=0, max_val=(E - 1) * FH,
                    skip_runtime_assert=True))
                w2offs.append(nc.s_assert_within(
                    e_s * H, min_val=0, max_val=(E - 1) * H,
                    skip_runtime_assert=True))

            rep_loop = tc.For_i(0, reps, name="reps")
            rep_loop.__enter__()
            j = 0
            for s in range(S):
                w1sb = w1pool.tile([128, HT, FH], _BF16, tag="w1sb")
                nc.sync.dma_start(w1sb[:],
                                  w1_r[:, :, bass.ds(w1offs[s], FH)])
                w2sb = w2pool.tile([128, FT2, H], _BF16, tag="w2sb")
                nc.sync.dma_start(w2sb[:],
                                  w2_r[:, :, bass.ds(w2offs[s], H)])
                for W in bodies[s]:
                    col = j * UNIT
                    xt_sb = xpool.tile([128, HT, W], _BF16, tag="xt")
                    nc.sync.dma_start(xt_sb[:],
                                      xt_r[:, :, col:col + W])
                    act_sb = apool.tile([128, FT2, W], _BF16, tag="act")
                    for f in range(FT2):
                        ps = ps1pool.tile([128, W], _F32, tag="ps1")
                        for h in range(HT):
                            nc.tensor.matmul(
                                ps[:],
                                w1sb[:, h, f * 128:(f + 1) * 128],
                                xt_sb[:, h],
                                start=(h == 0), stop=(h == HT - 1))
                        nc.scalar.activation(act_sb[:, f], ps[:], GELU_FUNC)
                    yt_sb = ypool.tile([128, HT, W], _F16, tag="yt")
                    for h in range(HT):
                        ps2 = ps2pool.tile([128, W], _F32, tag="ps2")
                        for f in range(FT2):
                            nc.tensor.matmul(
                                ps2[:],
                                w2sb[:, f, h * 128:(h + 1) * 128],
                                act_sb[:, f],
                                start=(f == 0), stop=(f == FT2 - 1))
                        nc.vector.tensor_copy(yt_sb[:, h], ps2[:])
                    nc.sync.dma_start(yt_r[:, :, col:col + W], yt_sb[:])
                    j += W // UNIT
            assert j == C
            rep_loop.__exit__(None, None, None)
    nc.compile()
    _cache[key] = nc
    return nc


def _make_inputs(x, w1, w2, schedule, reps=1):
    C, pattern, cells, units_of = schedule
    S = len(pattern)
    w1b = w1.astype(ml_dtypes.bfloat16)
    w2b = w2.astype(ml_dtypes.bfloat16)
    w1h_ = [np.ascontiguousarray(
        w1b[:, :, half * FH:(half + 1) * FH].transpose(1, 0, 2)
        .reshape(H, E * FH)) for half in range(2)]
    w2h_ = [np.ascontiguousarray(
        w2b[:, half * FH:(half + 1) * FH, :].transpose(1, 0, 2)
        .reshape(FH, E * H)) for half in range(2)]
    in_maps = []
    for pair in range(NPAIRS):
        xt = np.zeros((H, C * UNIT), ml_dtypes.bfloat16)
        for jj, (e, g, v) in enumerate(units_of[pair]):
            if v > 0:
                xt[:, jj * UNIT:jj * UNIT + v] = x[g:g + v].T
        meta = np.zeros((1, S + 1), np.int32)
        for s in range(S):
            meta[0, s] = cells[s][pair]
        meta[0, S] = reps
        for half in range(2):
            in_maps.append({"xt": xt, "w1": w1h_[half], "w2": w2h_[half],
                            "meta": meta})
    return in_maps


def _gather(results, schedule):
    C, pattern, cells, units_of = schedule
    out = np.zeros((T, H), np.float32)
    for pair in range(NPAIRS):
        ya = np.asarray(results[2 * pair]["yt"], np.float32)
        yb = np.asarray(results[2 * pair + 1]["yt"], np.float32)
        ys = ya + yb
        for jj, (e, g, v) in enumerate(units_of[pair]):
            if v > 0:
                out[g:g + v] = ys[:, jj * UNIT:jj * UNIT + v].T
    return out


def prepare(x, w1, w2, counts):
    """For test harness: compiled program + in_maps factory with a reps knob."""
    schedule = _schedule(counts)
    nc = _build(schedule[1])

    def make_in_maps(reps):
        return _make_inputs(x, w1, w2, schedule, reps=reps)

    return nc, make_in_maps


def kernel(permuted_local_hidden_states, weight1, weight2, tokens_per_expert):
    x = np.asarray(permuted_local_hidden_states, np.float32)
    w1 = np.asarray(weight1, np.float32)
    w2 = np.asarray(weight2, np.float32)
    counts = np.asarray(tokens_per_expert).astype(np.int64)

    schedule = _schedule(counts)
    nc = _build(schedule[1])
    in_maps = _make_inputs(x, w1, w2, schedule)
    res = run_bass_kernel_spmd(nc, in_maps, list(range(NCORES)))
    return _gather(res.results, schedule)


# revision 7
# speedup vs baseline: 1.0433x; 1.0433x over previous
"""Grouped MLP (MoE expert MLP, ragged token groups) on 8 TRN2 NeuronCores.

Strategy: token-parallel across 4 core-pairs; tensor-parallel (split of
the intermediate dim F) within each pair. Tokens are grouped contiguously
by expert; each expert's group is padded to a multiple of 256 tokens
("units"). Units are assigned to the 4 pairs by a greedy scheduler that
produces a single uniform slot pattern (slot = run of units computed with
one expert's weights) so all 8 cores run ONE fully static SPMD program:

  for slot s (static):  DMA this slot's expert weights (expert id is
      runtime data via a dynamic DRAM offset; weight pools are
      double-buffered so slot s+1 prefetches during slot s compute)
    for body b in slot (static, 512- or 256-token bodies):
      DMA xT [128, 8ht, W] -> fc1 (16 f-tiles x 8 h-tiles matmuls, W wide)
      -> Gelu -> fc2 (8 h-tiles x 16 f-tiles) -> yT fp16 partial -> DMA out

Everything is statically unrolled, so Tile overlaps all DMA with compute;
there are no dynamic-loop all-engine barriers in the steady state (the
only runtime loop is an outer `reps` loop used for timing).

Each core of a pair computes fc1/fc2 over its half of F and writes fp16
partial fc2 sums; the host adds the two halves and scatters to [T, H].
"""

import numpy as np
import ml_dtypes

import concourse.bass as bass
import concourse.mybir as mybir
import concourse.tile as tile
from concourse import bacc
from concourse.bass_utils import run_bass_kernel_spmd

# Problem shape (fixed by the task).
T, H, F, E = 16384, 1024, 4096, 8
NCORES = 8
NPAIRS = 4
UNIT = 256            # token unit (min matmul moving width)
HT = H // 128         # 8 h-tiles
FT2 = F // 2 // 128   # 16 f-tiles per core (half of F)
FH = F // 2           # 2048

_BF16 = mybir.dt.bfloat16
_F16 = mybir.dt.float16
_F32 = mybir.dt.float32
_I32 = mybir.dt.int32

GELU_FUNC = mybir.ActivationFunctionType.Gelu

_cache = {}


def _schedule(counts):
    """counts[E] -> (C, pattern, cells, units_of) for 4 uniform pairs.

    pattern: tuple of slot widths (in units), sum == C
    cells[s][g]: expert id for slot s on pair g
    units_of[g]: list of (expert, tok_start, valid) per chunk j (len C);
                 dummies are (e, -1, 0).
    """
    starts = np.concatenate([[0], np.cumsum(counts)])
    queues = {}
    n = 0
    for e in range(E):
        c = int(counts[e])
        q = []
        for off in range(0, c, UNIT):
            q.append((e, int(starts[e]) + off, min(UNIT, c - off)))
        if q:
            queues[e] = q
            n += len(q)
    if n == 0:
        queues[0] = [(0, -1, 0)]
        n = 1
    C = -(-n // NPAIRS)

    # Greedy slot pattern: each slot assigns (possibly repeated) experts
    # to the 4 pairs; width = what all assigned cells can fill.
    r = {e: len(q) for e, q in queues.items()}
    pattern, cells = [], []
    cap = C
    while cap > 0 and sum(r.values()) > 0:
        order = sorted([e for e in r if r[e] > 0], key=lambda e: -r[e])
        cell = [order[g % len(order)] for g in range(NPAIRS)]
        shares = {}
        for e in cell:
            shares[e] = shares.get(e, 0) + 1
        p = min(max(1, r[e] // shares[e]) for e in set(cell))
        p = max(1, min(p, cap))
        for e in cell:
            r[e] = max(0, r[e] - p)
        pattern.append(p)
        cells.append(cell)
        cap -= p
    if sum(r.values()) > 0:
        # Greedy failed (shouldn't for sane inputs): width-1 fallback.
        pattern, cells = [], []
        r = {e: len(q) for e, q in queues.items()}
        cap = C
        while cap > 0 and sum(r.values()) > 0:
            order = sorted([e for e in r if r[e] > 0], key=lambda e: -r[e])
            cell = [order[g % len(order)] for g in range(NPAIRS)]
            for e in cell:
                r[e] = max(0, r[e] - 1)
            pattern.append(1)
            cells.append(cell)
            cap -= 1
        assert sum(r.values()) == 0, "schedule fallback failed"
    if cap > 0:
        pattern.append(cap)
        cells.append(list(cells[-1]) if cells else [0] * NPAIRS)

    # Deal actual units to (pair, slot) cells.
    pos = {e: 0 for e in queues}
    units_of = [[] for _ in range(NPAIRS)]
    for s, p in enumerate(pattern):
        for g in range(NPAIRS):
            e = cells[s][g]
            for _ in range(p):
                q = queues.get(e, [])
                if pos.get(e, 0) < len(q):
                    units_of[g].append(q[pos[e]])
                    pos[e] += 1
                else:
                    units_of[g].append((e, -1, 0))
    for e, q in queues.items():
        assert pos[e] == len(q), f"unplaced units for expert {e}"
    return sum(pattern), tuple(pattern), cells, units_of


def _bodies(pattern):
    """Slot widths -> list of per-slot body widths (tokens)."""
    out = []
    for p in pattern:
        bl = [2 * UNIT] * (p // 2) + [UNIT] * (p % 2)
        out.append(bl)
    return out


def _build(pattern):
    import os
    wonce = bool(int(os.environ.get("K_WONCE", "0")))  # timing ablation
    xonce = bool(int(os.environ.get("K_XONCE", "0")))  # timing ablation
    key = (pattern, wonce, xonce)
    if key in _cache:
        return _cache[key]
    C = sum(pattern)
    S = len(pattern)
    bodies = _bodies(pattern)

    nc = bacc.Bacc("TRN2", target_bir_lowering=False, debug=False,
                   num_devices=NCORES)
    xt_d = nc.declare_dram_parameter("xt", [H, C * UNIT], _BF16,
                                     isOutput=False)
    w1_d = nc.declare_dram_parameter("w1", [H, E * FH], _BF16,
                                     isOutput=False)
    w2_d = nc.declare_dram_parameter("w2", [FH, E * H], _BF16,
                                     isOutput=False)
    meta_d = nc.declare_dram_parameter("meta", [1, S + 1], _I32,
                                       isOutput=False)
    yt_d = nc.declare_dram_parameter("yt", [H, C * UNIT], _F16,
                                     isOutput=True)

    xt_r = xt_d.rearrange("(ht p) m -> p ht m", p=128)
    w1_r = w1_d.rearrange("(ht p) m -> p ht m", p=128)
    w2_r = w2_d.rearrange("(ft p) m -> p ft m", p=128)
    yt_r = yt_d.rearrange("(ht p) m -> p ht m", p=128)

    with tile.TileContext(nc) as tc:
        with (
            tc.tile_pool(name="meta", bufs=1) as mpool,
            tc.tile_pool(name="w1", bufs=2) as w1pool,
            tc.tile_pool(name="w2", bufs=2) as w2pool,
            tc.tile_pool(name="x", bufs=2) as xpool,
            tc.tile_pool(name="act", bufs=2) as apool,
            tc.tile_pool(name="y", bufs=2) as ypool,
            tc.tile_pool(name="ps1", bufs=4, space="PSUM") as ps1pool,
            tc.tile_pool(name="ps2", bufs=4, space="PSUM") as ps2pool,
        ):
            mt = mpool.tile([1, S + 1], _I32)
            nc.sync.dma_start(mt[:], meta_d[:])
            reps = nc.values_load(mt[:1, S:S + 1], min_val=1, max_val=100000,
                                  skip_runtime_bounds_check=True)
            w1offs, w2offs = [], []
            for s in range(S):
                # skip_runtime_bounds_check: runtime assert traps kill the
                # axon/PJRT execution path.
                e_s = nc.values_load(mt[:1, s:s + 1], min_val=0,
                                     max_val=E - 1,
                                     skip_runtime_bounds_check=True)
                w1offs.append(nc.s_assert_within(
                    e_s * FH, min_val=0, max_val=(E - 1) * FH,
                    skip_runtime_assert=True))
                w2offs.append(nc.s_assert_within(
                    e_s * H, min_val=0, max_val=(E - 1) * H,
                    skip_runtime_assert=True))

            if wonce:
                w1sb0 = w1pool.tile([128, HT, FH], _BF16, tag="w1sb")
                nc.sync.dma_start(w1sb0[:],
                                  w1_r[:, :, bass.ds(w1offs[0], FH)])
                w2sb0 = w2pool.tile([128, FT2, H], _BF16, tag="w2sb")
                nc.sync.dma_start(w2sb0[:],
                                  w2_r[:, :, bass.ds(w2offs[0], H)])
            if xonce:
                xt_sb0 = xpool.tile([128, HT, 2 * UNIT], _BF16, tag="xt")
                nc.sync.dma_start(xt_sb0[:], xt_r[:, :, 0:2 * UNIT])

            rep_loop = tc.For_i(0, reps, name="reps")
            rep_loop.__enter__()
            j = 0
            for s in range(S):
                if wonce:
                    w1sb, w2sb = w1sb0, w2sb0
                else:
                    w1sb = w1pool.tile([128, HT, FH], _BF16, tag="w1sb")
                    nc.sync.dma_start(w1sb[:],
                                      w1_r[:, :, bass.ds(w1offs[s], FH)])
                    w2sb = w2pool.tile([128, FT2, H], _BF16, tag="w2sb")
                    nc.sync.dma_start(w2sb[:],
                                      w2_r[:, :, bass.ds(w2offs[s], H)])
                for W in bodies[s]:
                    col = j * UNIT
                    if xonce:
                        xt_sb = xt_sb0[:, :, :W] if W != 2 * UNIT else xt_sb0
                    else:
                        xt_sb = xpool.tile([128, HT, W], _BF16, tag="xt")
                        nc.sync.dma_start(xt_sb[:],
                                          xt_r[:, :, col:col + W])
                    act_sb = apool.tile([128, FT2, W], _BF16, tag="act")
                    for f in range(FT2):
                        ps = ps1pool.tile([128, W], _F32, tag="ps1")
                        for h in range(HT):
                            nc.tensor.matmul(
                                ps[:],
                                w1sb[:, h, f * 128:(f + 1) * 128],
                                xt_sb[:, h],
                                start=(h == 0), stop=(h == HT - 1))
                        nc.scalar.activation(act_sb[:, f], ps[:], GELU_FUNC)
                    yt_sb = ypool.tile([128, HT, W], _F16, tag="yt")
                    for h in range(HT):
                        ps2 = ps2pool.tile([128, W], _F32, tag="ps2")
                        for f in range(FT2):
                            nc.tensor.matmul(
                                ps2[:],
                                w2sb[:, f, h * 128:(h + 1) * 128],
                                act_sb[:, f],
                                start=(f == 0), stop=(f == FT2 - 1))
                        nc.vector.tensor_copy(yt_sb[:, h], ps2[:])
                    nc.sync.dma_start(yt_r[:, :, col:col + W], yt_sb[:])
                    j += W // UNIT
            assert j == C
            rep_loop.__exit__(None, None, None)
    nc.compile()
    _cache[key] = nc
    return nc


def _make_inputs(x, w1, w2, schedule, reps=1):
    C, pattern, cells, units_of = schedule
    S = len(pattern)
    w1b = w1.astype(ml_dtypes.bfloat16)
    w2b = w2.astype(ml_dtypes.bfloat16)
    w1h_ = [np.ascontiguousarray(
        w1b[:, :, half * FH:(half + 1) * FH].transpose(1, 0, 2)
        .reshape(H, E * FH)) for half in range(2)]
    w2h_ = [np.ascontiguousarray(
        w2b[:, half * FH:(half + 1) * FH, :].transpose(1, 0, 2)
        .reshape(FH, E * H)) for half in range(2)]
    in_maps = []
    for pair in range(NPAIRS):
        xt = np.zeros((H, C * UNIT), ml_dtypes.bfloat16)
        for jj, (e, g, v) in enumerate(units_of[pair]):
            if v > 0:
                xt[:, jj * UNIT:jj * UNIT + v] = x[g:g + v].T
        meta = np.zeros((1, S + 1), np.int32)
        for s in range(S):
            meta[0, s] = cells[s][pair]
        meta[0, S] = reps
        for half in range(2):
            in_maps.append({"xt": xt, "w1": w1h_[half], "w2": w2h_[half],
                            "meta": meta})
    return in_maps


def _gather(results, schedule):
    C, pattern, cells, units_of = schedule
    out = np.zeros((T, H), np.float32)
    for pair in range(NPAIRS):
        ya = np.asarray(results[2 * pair]["yt"], np.float32)
        yb = np.asarray(results[2 * pair + 1]["yt"], np.float32)
        ys = ya + yb
        for jj, (e, g, v) in enumerate(units_of[pair]):
            if v > 0:
                out[g:g + v] = ys[:, jj * UNIT:jj * UNIT + v].T
    return out


def prepare(x, w1, w2, counts):
    """For test harness: compiled program + in_maps factory with a reps knob."""
    schedule = _schedule(counts)
    nc = _build(schedule[1])

    def make_in_maps(reps):
        return _make_inputs(x, w1, w2, schedule, reps=reps)

    return nc, make_in_maps


def kernel(permuted_local_hidden_states, weight1, weight2, tokens_per_expert):
    x = np.asarray(permuted_local_hidden_states, np.float32)
    w1 = np.asarray(weight1, np.float32)
    w2 = np.asarray(weight2, np.float32)
    counts = np.asarray(tokens_per_expert).astype(np.int64)

    schedule = _schedule(counts)
    nc = _build(schedule[1])
    in_maps = _make_inputs(x, w1, w2, schedule)
    res = run_bass_kernel_spmd(nc, in_maps, list(range(NCORES)))
    return _gather(res.results, schedule)
